# revision 39
# baseline (speedup 1.0000x reference)
"""Bahdanau-style additive attention on 8 TRN2 NeuronCores.

score(n, l) = v . tanh(decoder_hidden[n] @ W_h.T + encoder_hiddens[n, l] @ W_s.T)
attn = softmax(mask(score));  context[n] = attn[n] @ encoder_hiddens[n]

Sharding: data-parallel over batch N=64 -> 8 batches per core, weights
replicated, no collectives.

The eh matmul (E @ W_s.T, 2 GFLOP/batch) dominates. The first F8=512 of
the 1024 contraction dims run as fp8e4m3 DoubleRow matmuls (two 256-deep
pair-chunks per (kc, lt), 2 elem/cycle moving) and the remaining 512 as
bf16; scale-neutral quantization (W*4, E/4 - the scale that minimizes
subnormal loss) lets both accumulate into one PSUM group. Error measured
host-side AND on HW: attn 1.92e-2 < 2e-2 gate, bit-stable across runs.

Device layouts (prepared host-side; h-pair p/slot layout for DoubleRow):
  e8T  [NB, P, 2, 2, L] e4m3 - fp8 pair-chunks of encoder (h dims 0:512)/4
  ebT  [NB, P, 4, L] bf16  - bf16 chunks of encoder (h dims 512:1024)
  eN   [NB, P, 8, H] bf16  - natural layout (context matmul, l on partitions)
  ws8  [P, 2, 2, H] e4m3   - (W_s.T[:512] * 4) pair-chunk layout
  wsb  [P, 4, H] bf16      - W_s.T[512:]
  whk  [P, KC, HC, P] bf16 - W_h.T pre-tiled so per-kc slices DMA as
                             contiguous per-partition runs (dh just-in-time)
  decT [P, HC, NB] bf16    - decoder shard transposed
  vcol [P, KC] bf16        - v reshaped so chunk c lives at [:, c]
  mneg [4, NB, QL] f32     - compact mask rows (-1e30 at PAD); rest of the
                             [P, NB, QL] SBUF tile memset to -1e30 once
Compute: f32 PSUM accumulation, f32 softmax. Score contraction for a whole
batch runs as one 32-matmul burst (4-way col-tiled quads x 8 kc) deferred
into the next batch's eh stream so tanh latency never stalls the PE; the
softmax exp is emitted right after the burst so it beats the next tanh
into the scalar FIFO, and the rest of the tail lands two eh groups later.

Preamble discipline (the Sync sequencer issues each dma_start serially at
~620ns): few big transfers, ordered ws8/e8T0 -> whk[0:2] -> (wsb, ebT0)
chunk pairs that batch 0's first two kc groups consume as they land; a
blocker DMA reading eb0's tail keeps later loads from stealing HBM
bandwidth. PE warmup matmuls cover the DMA window so the HAM clock gate
is 8/8 when the real stream starts.
"""

import os
import numpy as np
import ml_dtypes

N_CORES = 8
N, L, H = 64, 1024, 1024
NB = N // N_CORES  # batches per core
P = 128
F8 = 512           # h dims in fp8 DoubleRow (2 pair-chunks)
NP8 = F8 // 256    # DoubleRow pair-chunks
HCB = (H - F8) // P  # bf16 h chunks (6)
HC = H // P        # full h chunks (8) for the wh/dh path
KC = H // P        # output k chunks
LC = L // P
QL = L // 4        # 256; score quarter j lives at psum row 32j
QH = H // 4

NWARM = 64         # upfront PE warmup matmuls (HAM + preamble DMA cover)
NFILL_DR = 12      # fillers between kc01's fp8 and bf16 matmuls (DMA wait)
NFILL_KC0 = 6      # filler matmuls per chunk pair during DMA-paced kc01
NFILL_END = 16     # fillers before the last batch's score burst

_cache = {}

last_exec_time_ns = None
last_trace = None


def _build():
    import concourse.bass as bass
    import concourse.bacc as bacc
    import concourse.tile as tile
    from concourse import mybir

    f32 = mybir.dt.float32
    bf16 = mybir.dt.bfloat16
    fp8 = mybir.dt.float8e4
    TANH = mybir.ActivationFunctionType.Tanh
    EXP = mybir.ActivationFunctionType.Exp
    DR = mybir.MatmulPerfMode.DoubleRow

    nc = bacc.Bacc("TRN2", target_bir_lowering=False, debug=False,
                   num_devices=N_CORES)

    e8T = nc.dram_tensor("e8T", [NB, P, NP8, 2, L], fp8, kind="ExternalInput")
    ebT = nc.dram_tensor("ebT", [NB, P, HCB, L], bf16, kind="ExternalInput")
    eN = nc.dram_tensor("eN", [NB, P, LC, H], bf16, kind="ExternalInput")
    ws8 = nc.dram_tensor("ws8", [P, NP8, 2, H], fp8, kind="ExternalInput")
    wsb = nc.dram_tensor("wsb", [P, HCB, H], bf16, kind="ExternalInput")
    whk = nc.dram_tensor("whk", [P, KC, HC, P], bf16, kind="ExternalInput")
    decT = nc.dram_tensor("decT", [P, HC, NB], bf16, kind="ExternalInput")
    vcol = nc.dram_tensor("vcol", [P, KC], bf16, kind="ExternalInput")
    mneg = nc.dram_tensor("mneg", [4, NB, QL], f32, kind="ExternalInput")
    ctx_out = nc.dram_tensor("ctx", [NB, H], f32, kind="ExternalOutput")
    attn_out = nc.dram_tensor("attn", [NB, L], f32, kind="ExternalOutput")
    scrap = nc.dram_tensor("scrap", [1, 16], bf16, kind="ExternalOutput")

    with tile.TileContext(nc) as tc:
        with (
            tc.tile_pool(name="const", bufs=1) as cpool,
            tc.tile_pool(name="e8", bufs=2) as e8pool,
            tc.tile_pool(name="eb", bufs=2) as ebpool,
            tc.tile_pool(name="en", bufs=2) as enpool,
            tc.tile_pool(name="th", bufs=2) as thpool,
            tc.tile_pool(name="work", bufs=3) as wpool,
            tc.tile_pool(name="rows", bufs=2) as rpool,
            tc.tile_pool(name="ps", bufs=4, space=bass.MemorySpace.PSUM) as ppool,
            tc.tile_pool(name="ps1", bufs=1, space=bass.MemorySpace.PSUM) as ppool1,
            tc.tile_pool(name="psrow", bufs=2, space=bass.MemorySpace.PSUM) as prow,
            tc.tile_pool(name="pdh", bufs=1, space=bass.MemorySpace.PSUM) as pdh,
        ):
            # ---- preamble DMAs. The Sync sequencer issues each dma_start
            # serially (~620ns), so the critical path uses FEW BIG
            # transfers (each fans out over all 16 SDMA engines); a
            # blocker DMA reading the tail of eb0 stalls the sync FIFO so
            # later loads don't steal HBM bandwidth from the critical
            # set ----
            ws8_sb = cpool.tile([P, NP8, 2, H], fp8)
            nc.sync.dma_start(ws8_sb[:], ws8[:, :, :, :])
            e80_sb = e8pool.tile([P, NP8, 2, L], fp8, tag="e8")
            nc.sync.dma_start(e80_sb[:], e8T[0, :, :, :, :])
            dec_sb = cpool.tile([P, HC, NB], bf16)
            nc.sync.dma_start(dec_sb[:], decT[:, :, :])
            v_sb = cpool.tile([P, KC], bf16)
            nc.sync.dma_start(v_sb[:], vcol[:, :])
            wh_sb = cpool.tile([P, KC, HC, P], bf16)
            nc.sync.dma_start(wh_sb[:, 0:2, :, :], whk[:, 0:2, :, :])
            # batch-0's bf16 weights+encoder stream as (wsb, eb) chunk
            # pairs; kc0/kc1 chew each pair as it lands
            wsb_sb = cpool.tile([P, HCB, H], bf16)
            eb0_sb = ebpool.tile([P, HCB, L], bf16, tag="eb")
            for hcb in range(HCB):
                nc.sync.dma_start(wsb_sb[:, hcb, :], wsb[:, hcb, :])
                nc.sync.dma_start(eb0_sb[:, hcb, :], ebT[0, :, hcb, :])
            nc.sync.dma_start(scrap[:, :], eb0_sb[0:1, HCB - 1, 1008:1024])
            nc.sync.dma_start(wh_sb[:, 2:3, :, :], whk[:, 2:3, :, :])
            nc.sync.dma_start(wh_sb[:, 3:4, :, :], whk[:, 3:4, :, :])
            nc.sync.dma_start(wh_sb[:, 4:KC, :, :], whk[:, 4:KC, :, :])
            en0_sb = enpool.tile([P, LC, H], bf16, tag="en")
            nc.sync.dma_start(en0_sb[:], eN[0, :, :, :])

            # ---- warmup first: PE + the vector memset it needs go to the
            # head of their queues; remaining constants fill in behind ----
            warm_sb = cpool.tile([P, P], bf16)
            nc.vector.memset(warm_sb[:], 0.0)
            warm_ps = ppool1.tile([P, P], f32, tag="pc")

            def warm(k):
                for _ in range(k):
                    nc.tensor.matmul(warm_ps[:], warm_sb[:], warm_sb[:],
                                     start=True, stop=True)

            # covers engine ramp + preamble DMA so the HAM clock gate is
            # 8/8 when the real stream starts
            warm(NWARM)

            # mask tile: -1e30 everywhere, then 4 compact data rows
            mneg_sb = cpool.tile([P, NB, QL], f32)
            nc.vector.memset(mneg_sb[:], -1e30)
            for j in range(4):
                nc.sync.dma_start(mneg_sb[32 * j:32 * j + 1, :, :],
                                  mneg[j:j + 1, :, :])

            # ---- constants ----
            ones_sb = cpool.tile([P, 1], bf16)
            nc.vector.memset(ones_sb[:], 1.0)
            selbc_sb = cpool.tile([P, P], f32)
            nc.vector.memset(selbc_sb[:], 0.0)
            for j in range(4):
                nc.vector.memset(selbc_sb[32 * j:32 * j + 1, :], 1.0)
            dhT_sb = cpool.tile([P, KC, NB], f32)
            # scrub score psum slots once (quarters only write 4 rows; the
            # exp reads the full tile so stale bits must be finite)
            sc_init_a = prow.tile([P, QL], f32, tag="row")
            nc.vector.memset(sc_init_a[:], 0.0)
            sc_init_b = prow.tile([P, QL], f32, tag="row")
            nc.vector.memset(sc_init_b[:], 0.0)

            # dh accumulates into one persistent psum tile; per-kc regions
            # are independent so no WAR serialization between kc blocks
            dh_ps = pdh.tile([P, KC * NB], f32)

            def dh_block(kc):
                # dhT[k, n] for one kc column slice of W_h (whk[kc] arrives
                # just-in-time during batch 0's eh stream)
                for hc in range(HC):
                    nc.tensor.matmul(dh_ps[:, kc * NB:(kc + 1) * NB],
                                     wh_sb[:, kc, hc, :],
                                     dec_sb[:, hc, :],
                                     start=(hc == 0), stop=(hc == HC - 1))
                nc.vector.tensor_copy(dhT_sb[:, kc, :],
                                      dh_ps[:, kc * NB:(kc + 1) * NB])

            def emit_score_part(sc_ps, th_prev, kc_lo, kc_hi):
                # score[l] += v_kc . th[kc, :, l] for kc in [kc_lo, kc_hi),
                # 4-way col-tiled quarters -> psum rows {0,32,64,96}
                for kcs in range(kc_lo, kc_hi):
                    for j in range(4):
                        nc.tensor.matmul(
                            sc_ps[32 * j:32 * j + 1, :],
                            v_sb[:, kcs:kcs + 1],
                            th_prev[:, kcs, j * QL:(j + 1) * QL],
                            start=(kcs == 0), stop=(kcs == KC - 1),
                            tile_position=(0, 32 * j))

            def emit_score(ns, th_prev):
                sc_ps = prow.tile([P, QL], f32, tag="row")
                emit_score_part(sc_ps, th_prev, 0, KC)
                return sc_ps

            def emit_tail_a(n, sc_ps):
                # masked exp, emitted right after the score burst so it
                # lands ahead of the next tanh in the scalar FIFO
                sc_m = rpool.tile([P, QL], f32, tag="scrow")
                nc.vector.tensor_add(sc_m[:], sc_ps[:], mneg_sb[:, n, :])
                prob = rpool.tile([P, QL], f32, tag="prob")
                zs4 = wpool.tile([P, 1], f32, tag="z4")
                nc.scalar.activation(prob[:], sc_m[:], EXP, accum_out=zs4[:])
                return prob, zs4

            def emit_tail(n, prob, zs4, en_sb):
                # softmax normalization + attn out + context
                z_ps = ppool1.tile([P, 1], f32, tag="pc")
                nc.tensor.matmul(z_ps[:], selbc_sb[:], zs4[:],
                                 start=True, stop=True)
                rzb = wpool.tile([P, 1], f32, tag="rz")
                nc.vector.reciprocal(rzb[:], z_ps[:])
                arow_b = wpool.tile([P, QL], bf16, tag="arowb")
                nc.vector.tensor_scalar_mul(arow_b[:], prob[:], rzb[:])
                arow_f = rpool.tile([P, QL], f32, tag="arowf")
                nc.vector.tensor_scalar_mul(arow_f[:], prob[:], rzb[:])
                for j in range(4):
                    nc.sync.dma_start(
                        attn_out[n:n + 1, j * QL:(j + 1) * QL],
                        arow_f[32 * j:32 * j + 1, :])

                # transpose attn quarters -> columns via outer products
                ac_ps = ppool1.tile([P, LC], f32, tag="pc")
                for lc in range(LC):
                    j = lc // 2
                    nc.tensor.matmul(ac_ps[:, lc:lc + 1],
                                     arow_b[32 * j:32 * j + 1,
                                            (lc % 2) * P:(lc % 2 + 1) * P],
                                     ones_sb[32 * j:32 * j + 1, :],
                                     start=True, stop=True,
                                     tile_position=(32 * j, 0))
                acol = wpool.tile([P, LC], bf16, tag="acol")
                nc.vector.tensor_copy(acol[:], ac_ps[:])

                # context[n, h] = sum_l attn[l] E[l, h]; 4 col-tiled
                # h-quarters at psum rows 32j
                cx_ps = ppool1.tile([P, QH], f32, tag="pc")
                for lc in range(LC):
                    for j in range(4):
                        nc.tensor.matmul(
                            cx_ps[32 * j:32 * j + 1, :],
                            acol[:, lc:lc + 1],
                            en_sb[:, lc, j * QH:(j + 1) * QH],
                            start=(lc == 0), stop=(lc == LC - 1),
                            tile_position=(0, 32 * j))
                cx_row = rpool.tile([P, QH], f32, tag="cxrow")
                nc.vector.tensor_copy(cx_row[:], cx_ps[:])
                for j in range(4):
                    nc.sync.dma_start(ctx_out[n:n + 1, j * QH:(j + 1) * QH],
                                      cx_row[32 * j:32 * j + 1, :])

            # ---- fused per-batch pipeline ----
            pend_score = None  # (n, th_sb) awaiting the score burst
            pend_tail = None   # (n, sc_ps, en_sb) awaiting softmax+context
            for n in range(NB):
                if n == 0:
                    e8_sb, eb_sb, en_sb = e80_sb, eb0_sb, en0_sb
                else:
                    e8_sb = e8pool.tile([P, NP8, 2, L], fp8, tag="e8")
                    nc.sync.dma_start(e8_sb[:], e8T[n, :, :, :, :])
                    eb_sb = ebpool.tile([P, HCB, L], bf16, tag="eb")
                    nc.sync.dma_start(eb_sb[:], ebT[n, :, :, :])
                    en_sb = enpool.tile([P, LC, H], bf16, tag="en")
                    nc.sync.dma_start(en_sb[:], eN[n, :, :, :])

                th_sb = thpool.tile([P, KC, L], bf16, tag="th")
                if n == 0:
                    # kc0+kc1 together, streaming over (wsb, eb) chunk
                    # pairs as the preamble DMA delivers them
                    ps01 = [[ppool.tile([P, 512], f32, tag="ehps",
                                        name=f"ps01_{i}_{j}")
                             for j in range(2)] for i in range(2)]
                    for kc in range(2):
                        for c in range(NP8):
                            for lt in range(2):
                                nc.tensor.matmul(
                                    ps01[kc][lt][:],
                                    ws8_sb[:, c, :, kc * P:(kc + 1) * P],
                                    e8_sb[:, c, :, lt * 512:(lt + 1) * 512],
                                    start=(c == 0), stop=False,
                                    perf_mode=DR)
                    warm(NFILL_DR)
                    for hcb in range(HCB):
                        for kc in range(2):
                            for lt in range(2):
                                nc.tensor.matmul(
                                    ps01[kc][lt][:],
                                    wsb_sb[:, hcb, kc * P:(kc + 1) * P],
                                    eb_sb[:, hcb, lt * 512:(lt + 1) * 512],
                                    start=False, stop=(hcb == HCB - 1))
                        warm(NFILL_KC0)
                    for kc in range(2):
                        dh_block(kc)
                        for lt in range(2):
                            nc.scalar.activation(
                                th_sb[:, kc, lt * 512:(lt + 1) * 512],
                                ps01[kc][lt][:], TANH,
                                bias=dhT_sb[:, kc, n:n + 1])
                for kc in range(2 if n == 0 else 0, KC):
                    # bf16 chunks first, fp8 DoubleRow last: a resume after
                    # a score/tail interruption then pays only a fast bf16
                    # FWL reload, and the slow 256-col DR LDWEIGHTS hides
                    # under the bf16 stretch
                    eh_lt = [ppool.tile([P, 512], f32, tag="ehps",
                                        name=f"ehps_{kc}_{j}")
                             for j in range(2)]
                    for hcb in range(HCB):
                        for lt in range(2):
                            nc.tensor.matmul(
                                eh_lt[lt][:],
                                wsb_sb[:, hcb, kc * P:(kc + 1) * P],
                                eb_sb[:, hcb, lt * 512:(lt + 1) * 512],
                                start=(hcb == 0), stop=False)
                    for c in range(NP8):
                        for lt in range(2):
                            nc.tensor.matmul(
                                eh_lt[lt][:],
                                ws8_sb[:, c, :, kc * P:(kc + 1) * P],
                                e8_sb[:, c, :, lt * 512:(lt + 1) * 512],
                                start=False, stop=(c == NP8 - 1),
                                perf_mode=DR)
                    if n == 0 and kc <= 4:
                        dh_block(kc)
                        if kc == 4:
                            # whk is fully resident by now; finishing dh
                            # early keeps the late tanh's off the critical
                            # path at the batch-0/1 boundary
                            for kcl in range(5, KC):
                                dh_block(kcl)
                    for lt in range(2):
                        nc.scalar.activation(
                            th_sb[:, kc, lt * 512:(lt + 1) * 512],
                            eh_lt[lt][:], TANH,
                            bias=dhT_sb[:, kc, n:n + 1])
                    if kc == 1 and pend_score is not None:
                        ns, th_prev = pend_score
                        pend_score = None
                        sc_ps = emit_score(ns, th_prev)
                        prob, zs4 = emit_tail_a(ns, sc_ps)
                        pend_tail = (ns, prob, zs4, prev_en)
                    if kc == 3 and pend_tail is not None:
                        emit_tail(*pend_tail)
                        pend_tail = None
                    if n == NB - 1 and kc == KC - 2:
                        # last batch: most of the score burst overlaps the
                        # final eh groups; only the kc7 quads remain after
                        sc7_ps = prow.tile([P, QL], f32, tag="row")
                        emit_score_part(sc7_ps, th_sb, 0, KC - 2)
                    if n == NB - 1 and kc == KC - 1:
                        emit_score_part(sc7_ps, th_sb, KC - 2, KC - 1)
                pend_score = (n, th_sb)
                prev_en = en_sb
            # flush the last batch (nothing left to overlap with)
            warm(NFILL_END)
            ns, th_prev = pend_score
            emit_score_part(sc7_ps, th_prev, KC - 1, KC)
            prob, zs4 = emit_tail_a(ns, sc7_ps)
            emit_tail(ns, prob, zs4, prev_en)

    nc.compile()
    return nc


def kernel(decoder_hidden, encoder_hiddens, mask, W_h, W_s, v):
    global last_exec_time_ns, last_trace
    from concourse.bass_utils import run_bass_kernel_spmd

    bf16 = ml_dtypes.bfloat16
    e4 = ml_dtypes.float8_e4m3
    dec = np.asarray(decoder_hidden, np.float32)
    enc = np.asarray(encoder_hiddens, np.float32)
    msk = np.asarray(mask)
    W_h = np.asarray(W_h, np.float32)
    W_s = np.asarray(W_s, np.float32)
    v = np.asarray(v, np.float32)

    wsT = np.ascontiguousarray(W_s.T)  # [h, k]
    ws8 = np.ascontiguousarray(
        (wsT[:F8] * 4.0).astype(e4)
        .reshape(NP8, 2, P, H).transpose(2, 0, 1, 3))
    wsb = np.ascontiguousarray(
        wsT[F8:].astype(bf16).reshape(HCB, P, H).transpose(1, 0, 2))
    whT = np.ascontiguousarray(W_h.T)  # [h, k]
    whk = np.ascontiguousarray(
        whT.reshape(HC, P, KC, P).transpose(1, 2, 0, 3)).astype(bf16)
    vcol = np.ascontiguousarray(v.reshape(KC, P).T).astype(bf16)
    mneg_rows = np.where(msk, np.float32(-1e30), np.float32(0.0))  # [N, L]

    e8T_full = np.ascontiguousarray(
        (enc[..., :F8] * 0.25).astype(e4)
        .reshape(N, L, NP8, 2, P).transpose(0, 4, 2, 3, 1))  # [N,P,c,j,L]
    enc_bf = enc.astype(bf16)
    ebT_full = np.ascontiguousarray(
        enc_bf[..., F8:].reshape(N, L, HCB, P).transpose(0, 3, 2, 1))
    eN_full = np.ascontiguousarray(
        enc_bf.reshape(N, LC, P, H).transpose(0, 2, 1, 3))  # [N, P, LC, H]

    in_maps = []
    for c in range(N_CORES):
        s = slice(c * NB, (c + 1) * NB)
        in_maps.append({
            "e8T": e8T_full[s],
            "ebT": ebT_full[s],
            "eN": eN_full[s],
            "ws8": ws8,
            "wsb": wsb,
            "whk": whk,
            "decT": np.ascontiguousarray(
                dec[s].T.reshape(HC, P, NB).transpose(1, 0, 2)).astype(bf16),
            "vcol": vcol,
            "mneg": np.ascontiguousarray(
                mneg_rows[s].reshape(NB, 4, QL).transpose(1, 0, 2)),
        })

    if "nc" not in _cache:
        _cache["nc"] = _build()
    nc = _cache["nc"]

    trace = bool(int(os.environ.get("BASS_KERNEL_TRACE", "0")))
    res = run_bass_kernel_spmd(nc, in_maps, core_ids=list(range(N_CORES)),
                               trace=trace)
    last_exec_time_ns = res.exec_time_ns
    last_trace = res.instructions_and_trace

    context = np.concatenate([res.results[c]["ctx"] for c in range(N_CORES)], 0)
    attn_w = np.concatenate([res.results[c]["attn"] for c in range(N_CORES)], 0)
    return (context.astype(np.float32), attn_w.astype(np.float32))
